# revision 11
# baseline (speedup 1.0000x reference)
"""Trainium2 Bass kernel for adaptive-Tsit5 Neural ODE (nn_NeuralODE_1176821039669).

Strategy: data-parallel over 8 NeuronCores (batch 512 -> 64/core), feature-major
layout on-chip ([128 features x 64 batch] tiles). Fully unrolled masked solver:
15 intervals x 4 adaptive attempts, 6 MLP stages per attempt. fp32 throughout.

The step-size controller's pow ( (enorm^2)^-0.1 ) is computed with exponent/
mantissa bit manipulation + polynomials on the vector engine so the ACT table
set never switches (everything else uses the silu_and_others set).
"""

import numpy as np

# ---------------- problem constants (hardcoded per contract) ----------------
B, D, H, T = 512, 128, 256, 16
NCORES = 8
BC = B // NCORES  # 64 batch per core
NI = T - 1        # 15 intervals
MAX_STEPS = 4

C2, C3, C4, C5 = 0.161, 0.327, 0.9, 0.9800255409045097
A21 = 0.161
A31, A32 = -0.008480655492356989, 0.335480655492357
A41, A42, A43 = 2.8971530571054935, -6.359448489975075, 4.3622954328695815
A51, A52, A53, A54 = 5.325864828439257, -11.748883564062828, 7.4955393428898365, -0.09249506636175525
A61, A62, A63, A64, A65 = 5.86145544294642, -12.92096931784711, 8.159367898576159, -0.071584973281401, -0.028269050394068383
B1, B2, B3, B4, B5, B6 = 0.09646076681806523, 0.01, 0.4798896504144996, 1.379008574103742, -3.290069515436081, 2.324710524099774
E1, E2, E3, E4, E5, E6, E7 = (-0.001780011052226, -0.000816434459657, 0.007880878010262,
                              -0.144711007173263, 0.582357165452555, -0.458082105929187,
                              0.015151515151515152)
RTOL, ATOL = 1e-3, 1e-6
SAFETY, FMIN, FMAX = 0.9, 0.2, 10.0

# stage coefficient rows: stage s evaluates k_s at x_s = y + sum_j A[s][j]*v_j
STAGE_A = {
    2: [A21],
    3: [A31, A32],
    4: [A41, A42, A43],
    5: [A51, A52, A53, A54],
    6: [A61, A62, A63, A64, A65],
    7: [B1, B2, B3, B4, B5, B6],  # x_7 = y1
}
STAGE_C = {2: C2, 3: C3, 4: C4, 5: C5, 6: 1.0, 7: 1.0}
EVEC = [E1, E2, E3, E4, E5, E6, E7]

# ---------------- pow polynomial fits (host, numpy) ----------------


def _fit_poly_rel(f, lo, hi, deg, npts=4001):
    """Least-squares poly fit of f on [lo,hi] weighted for relative error.

    Returns coeffs highest-degree-first and max relative error."""
    x = np.linspace(lo, hi, npts)
    y = f(x)
    V = np.vander(x, deg + 1)  # columns x^deg ... x^0
    w = 1.0 / np.abs(y)
    A = V * w[:, None]
    b = np.ones_like(y)
    c, *_ = np.linalg.lstsq(A, b, rcond=None)
    err = np.max(np.abs((V @ c - y) / y))
    return c, err


_C0 = SAFETY * (128.0 ** 0.1)
_PM_COEF, _PM_ERR = _fit_poly_rel(lambda m: _C0 * m ** -0.1, 1.0, 2.0, 4)
_PF_COEF, _PF_ERR = _fit_poly_rel(lambda x: 2.0 ** x, -0.5, 0.5, 5)

# ---------------- bass kernel construction ----------------

_CACHE = {}


def _build(ts_host, n_intervals=NI, reps=1, fp16=True):
    import concourse.bacc as bacc
    import concourse.tile as tile
    from concourse import mybir

    F32 = mybir.dt.float32
    F16 = mybir.dt.float16
    MMD = F16 if fp16 else F32
    I32 = mybir.dt.int32
    ALU = mybir.AluOpType
    ACT = mybir.ActivationFunctionType

    f32 = lambda v: float(np.float32(v))
    tsf = [f32(v) for v in ts_host]
    t0 = tsf[0]
    dt0 = float(np.float32(tsf[1]) - np.float32(tsf[0]))

    nc = bacc.Bacc("TRN2", target_bir_lowering=False, debug=False)

    din = {}
    wspec = [("W1y", [128, H]), ("W1tb", [2, H]), ("W2a", [128, H]), ("W2b", [128, H]),
             ("b2r", [1, H]), ("W3a", [128, D]), ("W3b", [128, D]), ("b3r", [1, D])]
    for name, shape in wspec:
        din[name] = nc.dram_tensor(name, shape, MMD, kind="ExternalInput")
    din["y0T"] = nc.dram_tensor("y0T", [D, BC], F32, kind="ExternalInput")
    out_d = nc.dram_tensor("ysT", [n_intervals, D, BC], F32, kind="ExternalOutput")

    from contextlib import ExitStack
    with tile.TileContext(nc) as tc, ExitStack() as ctx:
        state = ctx.enter_context(tc.tile_pool(name="state", bufs=1))
        wpool = ctx.enter_context(tc.tile_pool(name="weights", bufs=1))
        vpool = ctx.enter_context(tc.tile_pool(name="vs", bufs=2))
        xpool = ctx.enter_context(tc.tile_pool(name="xs", bufs=2))
        hpool = ctx.enter_context(tc.tile_pool(name="hs", bufs=2))
        cpool = ctx.enter_context(tc.tile_pool(name="ctrl", bufs=2))
        bpool = ctx.enter_context(tc.tile_pool(name="bcast", bufs=2))
        pmm = ctx.enter_context(tc.tile_pool(name="pmm", bufs=2, space="PSUM"))
        pkk = ctx.enter_context(tc.tile_pool(name="pkk", bufs=2, space="PSUM"))
        prow = ctx.enter_context(tc.tile_pool(name="prow", bufs=1, space="PSUM"))
        pbc = ctx.enter_context(tc.tile_pool(name="pbc", bufs=1, space="PSUM"))
        if True:
            V = nc.vector
            G = nc.gpsimd
            S = nc.scalar
            PE = nc.tensor

            # ---- load inputs ----
            W1y = wpool.tile([128, H], MMD); nc.sync.dma_start(out=W1y, in_=din["W1y"].ap())
            W1tb = wpool.tile([2, H], MMD); nc.sync.dma_start(out=W1tb, in_=din["W1tb"].ap())
            W2a = wpool.tile([128, H], MMD); nc.sync.dma_start(out=W2a, in_=din["W2a"].ap())
            W2b = wpool.tile([128, H], MMD); nc.sync.dma_start(out=W2b, in_=din["W2b"].ap())
            B2r = wpool.tile([1, H], MMD); nc.sync.dma_start(out=B2r, in_=din["b2r"].ap())
            W3a = wpool.tile([128, D], MMD); nc.sync.dma_start(out=W3a, in_=din["W3a"].ap())
            W3b = wpool.tile([128, D], MMD); nc.sync.dma_start(out=W3b, in_=din["W3b"].ap())
            B3r = wpool.tile([1, D], MMD); nc.sync.dma_start(out=B3r, in_=din["b3r"].ap())

            Y = state.tile([D, BC], F32); nc.sync.dma_start(out=Y, in_=din["y0T"].ap())
            K1 = state.tile([D, BC], F32)
            Trow = state.tile([1, BC], F32); V.memset(Trow, t0)
            DTrow = state.tile([1, BC], F32); V.memset(DTrow, dt0)
            ones128 = state.tile([1, 128], F32); V.memset(ones128, 1.0)
            onesM = state.tile([128, 1], F32); V.memset(onesM, 1.0)
            ONESR = state.tile([1, BC], MMD); V.memset(ONESR, 1.0)
            TAUG = state.tile([2, BC], MMD); V.memset(TAUG, 1.0)
            TAUGS = {}
            for s in (2, 3, 4, 5, 6):
                TAUGS[s] = state.tile([2, BC], MMD, tag=f"taug{s}", name=f"taug{s}")
                V.memset(TAUGS[s], 1.0)

            # segment-constant tiles for batched MAC updates.
            # XL layout: segs 0..4 = ACC3..ACC7, seg 5 = ERR  (each 64 wide)
            # AjC[seg] holds A[s][j] for s = seg+3  (and E_j in seg 5)
            AC = []
            for j in range(1, 6):  # v_j has batched updates for j=1..5
                segs = []
                for s in range(3, 8):
                    row = STAGE_A[s]
                    segs.append(row[j - 1] if len(row) >= j else 0.0)
                segs.append(EVEC[j - 1])
                tl = wpool.tile([128, 6 * BC], F32, tag=f"A{j}C")
                for k, val in enumerate(segs):
                    V.memset(tl[:, k * BC:(k + 1) * BC], f32(val))
                AC.append(tl)

            import concourse.bass as bass

            def bview(t, n):
                """view a [P, BC] tile as [P, n, BC] broadcast along a new middle dim"""
                ap = t[:] if not isinstance(t, bass.AP) else t
                return bass.AP(tensor=ap.tensor, offset=ap.offset,
                               ap=[list(ap.ap[0]), [0, n], list(ap.ap[1])])

            def seg3(ap2d, n):
                """view a [P, n*BC] AP as [P, n, BC]"""
                return ap2d.rearrange("p (n b) -> p n b", b=BC)

            def emit_mlp(xa, trow):
                # trow is TAUG [2,BC] (row0: t_s, row1: ones)
                ph1 = pmm.tile([128, 2 * BC], F32, tag="ph1")
                PE.matmul(ph1[:, 0:BC], W1y[:, 0:128], xa, start=True, stop=False)
                PE.matmul(ph1[:, BC:2 * BC], W1y[:, 128:256], xa, start=False, stop=False)
                PE.matmul(ph1[:, 0:BC], W1tb[:, 0:128], trow, start=False, stop=False)
                PE.matmul(ph1[:, BC:2 * BC], W1tb[:, 128:256], trow, start=False, stop=True)
                H1 = hpool.tile([128, 2 * BC], MMD, tag="h1")
                S.activation(H1, ph1, ACT.Silu)
                ph2 = pmm.tile([128, 2 * BC], F32, tag="ph2")
                PE.matmul(ph2[:, 0:BC], W2a[:, 0:128], H1[:, 0:BC], start=True, stop=False)
                PE.matmul(ph2[:, BC:2 * BC], W2a[:, 128:256], H1[:, 0:BC], start=False, stop=False)
                PE.matmul(ph2[:, 0:BC], W2b[:, 0:128], H1[:, BC:2 * BC], start=False, stop=False)
                PE.matmul(ph2[:, BC:2 * BC], W2b[:, 128:256], H1[:, BC:2 * BC], start=False, stop=False)
                PE.matmul(ph2[:, 0:BC], B2r[:, 0:128], ONESR, start=False, stop=False)
                PE.matmul(ph2[:, BC:2 * BC], B2r[:, 128:256], ONESR, start=False, stop=True)
                H2 = hpool.tile([128, 2 * BC], MMD, tag="h2")
                S.activation(H2, ph2, ACT.Silu)
                pk = pkk.tile([128, BC], F32, tag="pk")
                PE.matmul(pk, W3a, H2[:, 0:BC], start=True, stop=False)
                PE.matmul(pk, W3b, H2[:, BC:2 * BC], start=False, stop=False)
                PE.matmul(pk, B3r, ONESR, start=False, stop=True)
                return pk

            def bc_row(row, tag):
                """broadcast [1,BC] row across 128 partitions via PE outer product"""
                pb = pbc.tile([128, BC], F32, tag="pbc")
                PE.matmul(pb, ones128, row, start=True, stop=True)
                out = bpool.tile([128, BC], F32, tag=tag)
                S.activation(out, pb, ACT.Copy)
                return out

            # ---- k1 init: k1 = f(t0, y0) ----
            Y16 = state.tile([D, BC], MMD)
            V.tensor_copy(out=Y16, in_=Y)
            V.memset(TAUG[0:1, :], t0)
            pk0 = emit_mlp(Y16, TAUG)
            S.activation(K1, pk0, ACT.Copy)

            # pow polynomial coefficients (highest-first)
            pmc = [f32(c) for c in _PM_COEF]
            pfc = [f32(c) for c in _PF_COEF]

            for rep in range(reps):
              for interval in range(n_intervals):
                t1 = tsf[interval + 1]
                thr = float(np.float32(t1) - np.float32(1e-8))
                for attempt in range(MAX_STEPS):
                    last_step = (rep == reps - 1) and (interval == n_intervals - 1) and (attempt == MAX_STEPS - 1)
                    # ---- control A: nd, dtc, bc_dtc ----
                    NDf = cpool.tile([1, BC], F32, tag="ndf")
                    V.tensor_scalar(out=NDf, in0=Trow, scalar1=thr, scalar2=None, op0=ALU.is_lt)
                    NDi = cpool.tile([1, BC], I32, tag="ndi")
                    V.tensor_copy(out=NDi, in_=NDf)
                    U2 = cpool.tile([1, BC], F32, tag="u2")
                    V.tensor_scalar(out=U2, in0=Trow, scalar1=-1.0, scalar2=t1, op0=ALU.mult, op1=ALU.add)
                    DTC = cpool.tile([1, BC], F32, tag="dtc")
                    V.tensor_tensor(out=DTC, in0=DTrow, in1=U2, op=ALU.min)
                    BCDT = bc_row(DTC, "bcdt")

                    # t rows for stages (t + C_s*dtc); stages 6,7 share.
                    # persistent taug tiles: row0 = t_s (rewritten), row1 = ones
                    trows = {}
                    for s, cs in [(2, C2), (3, C3), (4, C4), (5, C5), (6, 1.0)]:
                        tr = TAUGS[s]
                        V.scalar_tensor_tensor(out=tr[0:1, :], in0=DTC, scalar=f32(cs), in1=Trow,
                                               op0=ALU.mult, op1=ALU.add)
                        trows[s] = tr
                    trows[7] = trows[6]

                    # ---- stage 2 ----
                    V1 = vpool.tile([D, BC], F32, tag="v1")
                    V.tensor_tensor(out=V1, in0=BCDT, in1=K1, op=ALU.mult)
                    X2 = xpool.tile([D, BC], MMD, tag="x2")
                    V.scalar_tensor_tensor(out=X2, in0=V1, scalar=f32(A21), in1=Y,
                                           op0=ALU.mult, op1=ALU.add)
                    # XL init: ACC3..ACC7 = A_s1*v1 + y ; ERR = E1*v1
                    XL = xpool.tile([D, 6 * BC], F32, tag="xl")
                    V.tensor_tensor(out=seg3(XL[:, 0:5 * BC], 5), in0=bview(V1, 5), in1=seg3(AC[0][:, 0:5 * BC], 5), op=ALU.mult)
                    V.tensor_tensor(out=seg3(XL[:, 0:5 * BC], 5), in0=seg3(XL[:, 0:5 * BC], 5), in1=bview(Y, 5), op=ALU.add)
                    V.tensor_scalar(out=XL[:, 5 * BC:6 * BC], in0=V1, scalar1=f32(E1), scalar2=None, op0=ALU.mult)

                    pk2 = emit_mlp(X2, trows[2])

                    Vs = {1: V1}
                    Xcur = {2: X2}
                    for s in range(2, 8):
                        # v_s from this stage's psum k
                        pk = pk2 if s == 2 else Xcur["pk"]
                        Vss = vpool.tile([D, BC], F32, tag=f"v{s}")
                        V.tensor_tensor(out=Vss, in0=pk, in1=BCDT, op=ALU.mult)
                        Vs[s] = Vss
                        if s == 7:
                            K7 = vpool.tile([D, BC], F32, tag="k7")
                            S.activation(K7, pk, ACT.Copy)
                            V.scalar_tensor_tensor(out=XL[:, 5 * BC:6 * BC], in0=Vss, scalar=f32(E7),
                                                   in1=XL[:, 5 * BC:6 * BC], op0=ALU.mult, op1=ALU.add)
                            break
                        # batched MAC update of remaining ACC segs + ERR with v_s (j=s)
                        if s <= 5:
                            lo = (s - 1) * BC  # exclude segs for stages <= s+1 (crit handled below)
                            n = 6 - s
                            TMP = xpool.tile([D, 6 * BC], F32, tag="tmpmac")
                            V.tensor_tensor(out=seg3(TMP[:, lo:6 * BC], n + 1), in0=bview(Vss, n + 1),
                                            in1=seg3(AC[s - 1][:, lo:6 * BC], n + 1), op=ALU.mult)
                            V.tensor_tensor(out=XL[:, lo:6 * BC], in0=XL[:, lo:6 * BC],
                                            in1=TMP[:, lo:6 * BC], op=ALU.add)
                        else:  # s == 6: only ERR
                            V.scalar_tensor_tensor(out=XL[:, 5 * BC:6 * BC], in0=Vss, scalar=f32(EVEC[5]),
                                                   in1=XL[:, 5 * BC:6 * BC], op0=ALU.mult, op1=ALU.add)
                        # critical: X_{s+1} = A_{s+1,s}*v_s + ACC_{s+1}
                        nxt = s + 1
                        Xn = xpool.tile([D, BC], F32 if nxt == 7 else MMD, tag=f"x{nxt}")
                        V.scalar_tensor_tensor(out=Xn, in0=Vss, scalar=f32(STAGE_A[nxt][s - 1]),
                                               in1=XL[:, (nxt - 3) * BC:(nxt - 2) * BC],
                                               op0=ALU.mult, op1=ALU.add)
                        Xcur[nxt] = Xn
                        if nxt == 7 and fp16:
                            Xn16 = xpool.tile([D, BC], MMD, tag="x7m")
                            S.activation(Xn16, Xn, ACT.Copy)
                            Xcur["pk"] = emit_mlp(Xn16, trows[nxt])
                        else:
                            Xcur["pk"] = emit_mlp(Xn, trows[nxt])
                        # scale/reciprocal prep overlapped with stage 7 compute
                        if nxt == 7:
                            Y1 = Xn
                            AY = cpool.tile([D, BC], F32, tag="ay")
                            V.tensor_scalar(out=AY.bitcast(I32), in0=Y.bitcast(I32),
                                            scalar1=0x7FFFFFFF, scalar2=None, op0=ALU.bitwise_and)
                            AY1 = cpool.tile([D, BC], F32, tag="ay1")
                            V.tensor_scalar(out=AY1.bitcast(I32), in0=Y1.bitcast(I32),
                                            scalar1=0x7FFFFFFF, scalar2=None, op0=ALU.bitwise_and)
                            MX = cpool.tile([D, BC], F32, tag="mx")
                            V.tensor_tensor(out=MX, in0=AY, in1=AY1, op=ALU.max)
                            SC = cpool.tile([D, BC], F32, tag="sc")
                            V.tensor_scalar(out=SC, in0=MX, scalar1=f32(RTOL), scalar2=f32(ATOL),
                                            op0=ALU.mult, op1=ALU.add)
                            RSC = cpool.tile([D, BC], F32, tag="rsc")
                            V.reciprocal(out=RSC, in_=SC)

                    # ---- control B ----
                    Q = cpool.tile([D, BC], F32, tag="q")
                    V.tensor_tensor(out=Q, in0=XL[:, 5 * BC:6 * BC], in1=RSC, op=ALU.mult)
                    Q2 = cpool.tile([D, BC], F32, tag="q2")
                    S.activation(Q2, Q, ACT.Square)
                    pssq = prow.tile([1, BC], F32, tag="pssq")
                    PE.matmul(pssq, onesM, Q2, start=True, stop=True)
                    Srow = cpool.tile([1, BC], F32, tag="srow")
                    S.activation(Srow, pssq, ACT.Copy)

                    # accept mask m = (ssq <= 128) & nd ; tn = t + m*dtc
                    ACCf = cpool.tile([1, BC], F32, tag="accf")
                    V.tensor_scalar(out=ACCf, in0=Srow, scalar1=128.0, scalar2=None, op0=ALU.is_le)
                    Mf = cpool.tile([1, BC], F32, tag="mf")
                    V.tensor_tensor(out=Mf, in0=ACCf, in1=NDf, op=ALU.mult)
                    Mi = cpool.tile([1, BC], I32, tag="mi")
                    V.tensor_copy(out=Mi, in_=Mf)
                    pbm = pbc.tile([128, BC], F32, tag="pbc")
                    PE.matmul(pbm, ones128, Mf, start=True, stop=True)
                    BCMi = bpool.tile([128, BC], I32, tag="bcmi")
                    S.activation(BCMi, pbm, ACT.Copy)
                    MD = cpool.tile([1, BC], F32, tag="md")
                    V.tensor_tensor(out=MD, in0=DTC, in1=Mf, op=ALU.mult)
                    V.tensor_tensor(out=Trow, in0=Trow, in1=MD, op=ALU.add)
                    V.copy_predicated(out=Y, mask=BCMi, data=Y1)
                    V.copy_predicated(out=K1, mask=BCMi, data=K7)

                    if last_step:
                        nc.sync.dma_start(out=out_d.ap()[interval], in_=Y)
                        break
                    if attempt == MAX_STEPS - 1:
                        nc.sync.dma_start(out=out_d.ap()[interval], in_=Y)

                    # ---- factor = clip(0.9*(ssq/128)^-0.1, 0.2, 10) via bit pow ----
                    Sb = Srow.bitcast(I32)
                    Eb = cpool.tile([1, BC], I32, tag="eb")
                    V.tensor_scalar(out=Eb, in0=Sb, scalar1=0x7F800000, scalar2=None, op0=ALU.bitwise_and)
                    Ef = cpool.tile([1, BC], F32, tag="ef")
                    V.tensor_copy(out=Ef, in_=Eb)
                    U = cpool.tile([1, BC], F32, tag="u")
                    V.tensor_scalar(out=U, in0=Ef, scalar1=f32(-0.1 / (2.0 ** 23)), scalar2=12.7,
                                    op0=ALU.mult, op1=ALU.add)
                    Mb = cpool.tile([1, BC], I32, tag="mb")
                    V.tensor_scalar(out=Mb, in0=Sb, scalar1=0x007FFFFF, scalar2=0x3F800000,
                                    op0=ALU.bitwise_and, op1=ALU.bitwise_or)
                    Mfl = Mb.bitcast(F32)
                    Ii = cpool.tile([1, BC], I32, tag="ii")
                    V.tensor_copy(out=Ii, in_=U)  # f32 -> i32 round-to-nearest-even
                    If = cpool.tile([1, BC], F32, tag="if")
                    V.tensor_copy(out=If, in_=Ii)
                    FR = cpool.tile([1, BC], F32, tag="fr")
                    V.tensor_tensor(out=FR, in0=U, in1=If, op=ALU.subtract)
                    IB = cpool.tile([1, BC], I32, tag="ib")
                    V.tensor_scalar(out=IB, in0=Ii, scalar1=127, scalar2=None, op0=ALU.add)
                    IB2 = cpool.tile([1, BC], I32, tag="ib2")
                    V.tensor_scalar(out=IB2, in0=IB, scalar1=23, scalar2=None, op0=ALU.logical_shift_left)
                    P2I = IB2.bitcast(F32)

                    # pm chain on gpsimd (mantissa poly), pf chain on vector
                    PMW = cpool.tile([1, BC], F32, tag="pmw")
                    V.tensor_scalar(out=PMW, in0=Mfl, scalar1=pmc[0], scalar2=None, op0=ALU.mult)
                    for ci in range(1, len(pmc) - 1):
                        V.scalar_tensor_tensor(out=PMW, in0=PMW, scalar=pmc[ci], in1=Mfl,
                                               op0=ALU.add, op1=ALU.mult)
                    PFW = cpool.tile([1, BC], F32, tag="pfw")
                    V.tensor_scalar(out=PFW, in0=FR, scalar1=pfc[0], scalar2=None, op0=ALU.mult)
                    for ci in range(1, len(pfc) - 1):
                        V.scalar_tensor_tensor(out=PFW, in0=PFW, scalar=pfc[ci], in1=FR,
                                               op0=ALU.add, op1=ALU.mult)
                    GA = cpool.tile([1, BC], F32, tag="ga")
                    V.scalar_tensor_tensor(out=GA, in0=PMW, scalar=pmc[-1], in1=P2I,
                                           op0=ALU.add, op1=ALU.mult)
                    GB = cpool.tile([1, BC], F32, tag="gb")
                    V.scalar_tensor_tensor(out=GB, in0=PFW, scalar=pfc[-1], in1=GA,
                                           op0=ALU.add, op1=ALU.mult)
                    FACTOR = cpool.tile([1, BC], F32, tag="factor")
                    V.tensor_scalar(out=FACTOR, in0=GB, scalar1=f32(FMIN), scalar2=f32(FMAX),
                                    op0=ALU.max, op1=ALU.min)
                    DF = cpool.tile([1, BC], F32, tag="df")
                    V.tensor_tensor(out=DF, in0=DTC, in1=FACTOR, op=ALU.mult)
                    V.copy_predicated(out=DTrow, mask=NDi, data=DF)

    nc.finalize()
    return nc, din


def _prep_inputs(inputs, ts, W1, b1, W2, b2, W3, b3, fp16=True):
    f = np.float16 if fp16 else np.float32
    shared = dict(
        W1y=np.ascontiguousarray(W1[:128]).astype(f),
        W1tb=np.ascontiguousarray(np.stack([W1[128], b1], 0)).astype(f),
        W2a=np.ascontiguousarray(W2[:128]).astype(f),
        W2b=np.ascontiguousarray(W2[128:]).astype(f),
        b2r=np.ascontiguousarray(b2[None, :]).astype(f),
        W3a=np.ascontiguousarray(W3[:128]).astype(f),
        W3b=np.ascontiguousarray(W3[128:]).astype(f),
        b3r=np.ascontiguousarray(b3[None, :]).astype(f),
    )
    in_maps = []
    for c in range(NCORES):
        m = dict(shared)
        m["y0T"] = np.ascontiguousarray(inputs[c * BC:(c + 1) * BC].T, np.float32)
        in_maps.append(m)
    return in_maps


def kernel(inputs, ts, W1, b1, W2, b2, W3, b3, n_intervals=NI, trace=False, fp16=True):
    from concourse.bass_utils import run_bass_kernel_spmd

    inputs = np.asarray(inputs, np.float32)
    ts = np.asarray(ts, np.float32)
    key = (n_intervals, fp16, tuple(float(v) for v in ts))
    if key not in _CACHE:
        _CACHE[key] = _build(ts, n_intervals, fp16=fp16)
    nc, _ = _CACHE[key]

    in_maps = _prep_inputs(inputs, ts, np.asarray(W1, np.float32), np.asarray(b1, np.float32),
                           np.asarray(W2, np.float32), np.asarray(b2, np.float32),
                           np.asarray(W3, np.float32), np.asarray(b3, np.float32), fp16=fp16)
    res = run_bass_kernel_spmd(nc, in_maps, core_ids=list(range(NCORES)), trace=trace)
    out = np.empty((B, T, D), np.float32)
    for c in range(NCORES):
        ysT = res.results[c]["ysT"]  # [n_intervals, D, BC]
        out[c * BC:(c + 1) * BC, 0, :] = inputs[c * BC:(c + 1) * BC]
        out[c * BC:(c + 1) * BC, 1:n_intervals + 1, :] = ysT.transpose(2, 0, 1)
    if n_intervals < NI:
        out = out[:, :n_intervals + 1, :]
    kernel.last_result = res
    return out
